# revision 5
# baseline (speedup 1.0000x reference)
"""Chamfer loss kernel for Trainium2 (Bass/Tile), 8 NeuronCores.

Problem: predicted/expected [M=8, N=4096, D=64] fp32; per batch element m:
    dist[n,k] = ||a_n||^2 + ||b_k||^2 - 2 a_n.b_k   (clamped at 0)
    loss[m] = sum_n min_k dist + sum_k min_n dist

Sharding: pure data parallel over m — one batch element per core.

Per-core algorithm:
  - Build transposed operands aT/bT [D, N] in fp32r via PE transposes.
  - Append a 66th/67th contraction row pair: ones on the a side,
    (-|b|^2/2) split hi/lo on the b side, so the K=66 matmul produces
    psum = a.b - |b|^2/2 directly.
  - ACT copies each psum batch to SBUF fp16 while adding a per-partition
    bias of -|a|^2/2, giving copy_t = -dist/2.  Both chamfer directions
    are then max-reductions of copy_t:
      * rowmax (min over k): DVE tensor_scalar max-accum (4x mode)
      * colmax (min over n): DVE tensor_max accumulation (2x mode)
  - Epilogue: relu(-2*max) (= clamped min distance) summed per partition
    via ACT accum, cross-partition summed via a ones-matmul.
"""

from contextlib import ExitStack

import numpy as np

import concourse.bacc as bacc
import concourse.bass_utils as bass_utils
import concourse.mybir as mybir
import concourse.tile as tile
from concourse.masks import make_identity

M, N, D = 8, 4096, 64
P = 128  # partitions / a-row tile
CH = 512  # b-column chunk (one psum bank)
GRP = 2048  # columns per psum batch (4 banks)
NT = N // P  # 32 a-row tiles
NG = N // GRP  # 2 column groups per row tile
KAUG = D + 2  # contraction rows incl. ones x (-b2/2) hi/lo

F32 = mybir.dt.float32
F32R = mybir.dt.float32r
F16 = mybir.dt.float16
NEG_BIG = -3.0e38  # fp32 immediates
F16_NEG = -60000.0  # fp16-safe "-inf"
ALU_MAX = mybir.AluOpType.max


def _build_chamfer(ctx, nc, tc, pred, exp, out):
    fx = ctx.enter_context(tc.tile_pool(name="fx", bufs=1))
    work = ctx.enter_context(tc.tile_pool(name="work", bufs=2))
    cpool = ctx.enter_context(tc.tile_pool(name="cp", bufs=3))
    spool = ctx.enter_context(tc.tile_pool(name="sp", bufs=3))
    ps = ctx.enter_context(tc.tile_pool(name="ps", bufs=2, space="PSUM"))

    ident = fx.tile([P, P], F32, tag="ident")
    make_identity(nc, ident)

    # ---- load inputs in natural layout [128, 32, 64] ----
    a_nat = fx.tile([P, NT, D], F32, tag="a_nat")
    b_nat = fx.tile([P, NT, D], F32, tag="b_nat")
    nc.sync.dma_start(out=a_nat, in_=pred.rearrange("(c p) d -> p c d", p=P))
    nc.sync.dma_start(out=b_nat, in_=exp.rearrange("(c p) d -> p c d", p=P))

    # ---- squared norms ----
    # a2: keep in column layout [128, 32] as the ACT bias source (-a2/2).
    sq_a = work.tile([P, NT * D], F32, tag="sq")
    nc.scalar.square(sq_a, a_nat.rearrange("p c d -> p (c d)"))
    a2 = fx.tile([P, NT], F32, tag="a2")
    nc.vector.reduce_sum(a2, sq_a.rearrange("p (c d) -> p c d", d=D),
                         axis=mybir.AxisListType.X)
    neg_a2_half = fx.tile([P, NT], F32, tag="na2")
    nc.scalar.mul(neg_a2_half, a2, -0.5)

    # b2 likewise, then transpose to a row and split hi/lo for fp32r.
    sq_b = work.tile([P, NT * D], F32, tag="sq")
    nc.scalar.square(sq_b, b_nat.rearrange("p c d -> p (c d)"))
    b2 = fx.tile([P, NT], F32, tag="b2")
    nc.vector.reduce_sum(b2, sq_b.rearrange("p (c d) -> p c d", d=D),
                         axis=mybir.AxisListType.X)
    ps_b2 = ps.tile([NT, P], F32, tag="mm")
    nc.tensor.transpose(ps_b2, b2, ident)
    nb2h = fx.tile([NT, P], F32, tag="nb2h")  # -b2/2 (fp32 exact)
    nc.scalar.mul(nb2h, ps_b2, -0.5)
    nb2h_r = fx.tile([NT, P], F32R, tag="nb2hr")  # fp32r hi part
    nc.scalar.copy(nb2h_r, nb2h)
    nb2l_r = fx.tile([NT, P], F32R, tag="nb2lr")  # residual, fp32r
    nc.vector.tensor_sub(nb2l_r, nb2h, nb2h_r.bitcast(F32))

    # ---- transposed fp32r operands [66, 4096] ----
    aT = fx.tile([KAUG, N], F32R, tag="aT")
    bT = fx.tile([KAUG, N], F32R, tag="bT")
    for src, dst in ((a_nat, aT), (b_nat, bT)):
        for g in range(2):
            pt = ps.tile([D, 16 * P], F32, tag="mm")
            for c in range(16):
                nc.tensor.transpose(
                    pt[:, c * P:(c + 1) * P], src[:, 16 * g + c, :], ident
                )
            nc.scalar.copy(dst[0:D, g * 16 * P:(g + 1) * 16 * P], pt)
    # aug rows: a side = ones/ones, b side = (-b2/2) hi/lo
    # (memset can't target float32r — stage fp32 ones, round via ACT copy)
    ones_row = work.tile([2, N], F32, tag="ones_row")
    nc.vector.memset(ones_row, 1.0)
    nc.scalar.copy(aT[D:D + 2, :], ones_row)
    nc.sync.dma_start(
        out=bT[D:D + 1, :].rearrange("o (c f) -> o c f", c=NT), in_=nb2h_r
    )
    nc.sync.dma_start(
        out=bT[D + 1:D + 2, :].rearrange("o (c f) -> o c f", c=NT), in_=nb2l_r
    )

    # ---- accumulators ----
    colmax = [
        fx.tile([P, N], F16, tag=f"colmax{i}", name=f"colmax{i}") for i in range(2)
    ]
    nc.vector.memset(colmax[0], F16_NEG)
    rowparts = fx.tile([P, NT, NG], F32, tag="rowparts")

    # ---- main loop ----
    for i in range(NT):
        lhsT = aT[:, i * P:(i + 1) * P]
        src_cm = colmax[i % 2]
        dst_cm = colmax[(i + 1) % 2]
        for g in range(NG):
            pt = ps.tile([P, GRP], F32, tag="mm")
            for u in range(GRP // CH):
                rhs = bT[:, g * GRP + u * CH:g * GRP + (u + 1) * CH]
                nc.tensor.matmul(
                    pt[:, u * CH:(u + 1) * CH], lhsT, rhs, start=True, stop=True
                )
            ct = cpool.tile([P, GRP], F16, tag="ct")
            nc.scalar.activation(
                out=ct, in_=pt, func=mybir.ActivationFunctionType.Identity,
                bias=neg_a2_half[:, i:i + 1], scale=1.0,
            )
            nc.vector.tensor_max(
                dst_cm[:, g * GRP:(g + 1) * GRP], ct, src_cm[:, g * GRP:(g + 1) * GRP]
            )
            sc = spool.tile([P, GRP], F16, tag="sc")
            nc.vector.tensor_scalar(
                out=sc, in0=ct, scalar1=NEG_BIG, scalar2=None,
                op0=ALU_MAX, op1=ALU_MAX, accum_out=rowparts[:, i, g:g + 1],
            )

    colmax_fin = colmax[NT % 2]

    # ---- epilogue: loss_ab from rowparts ----
    rmax = fx.tile([P, NT], F32, tag="rmax")
    nc.vector.tensor_max(rmax, rowparts[:, :, 0], rowparts[:, :, 1])
    relu_r = fx.tile([P, NT], F32, tag="relu_r")
    row_sum = fx.tile([P, 1], F32, tag="row_sum")
    nc.scalar.activation(
        out=relu_r, in_=rmax, func=mybir.ActivationFunctionType.Relu,
        scale=-2.0, accum_out=row_sum,
    )

    # ---- epilogue: loss_ba from colmax (cross-partition via PE transpose) ----
    col32 = fx.tile([P, N], F32, tag="col32")
    nc.vector.tensor_copy(col32, colmax_fin)
    colT = fx.tile([P, NT], F32, tag="colT")
    for gr in range(8):
        pt = ps.tile([P, 4 * P], F32, tag="mm")
        for u in range(4):
            blk = 4 * gr + u
            nc.tensor.transpose(
                pt[:, u * P:(u + 1) * P], col32[:, blk * P:(blk + 1) * P], ident
            )
        nc.vector.reduce_max(
            colT[:, 4 * gr:4 * gr + 4],
            pt.rearrange("p (u f) -> p u f", u=4),
            axis=mybir.AxisListType.X,
        )
    relu_c = fx.tile([P, NT], F32, tag="relu_c")
    col_sum = fx.tile([P, 1], F32, tag="col_sum")
    nc.scalar.activation(
        out=relu_c, in_=colT, func=mybir.ActivationFunctionType.Relu,
        scale=-2.0, accum_out=col_sum,
    )

    # ---- total: cross-partition sum via ones-matmul ----
    both = fx.tile([P, 1], F32, tag="both")
    nc.vector.tensor_add(both, row_sum, col_sum)
    ones = fx.tile([P, 1], F32, tag="ones")
    nc.vector.memset(ones, 1.0)
    ps_f = ps.tile([1, 1], F32, tag="mm")
    nc.tensor.matmul(ps_f, both, ones, start=True, stop=True)
    res = fx.tile([1, 1], F32, tag="res")
    nc.scalar.copy(res, ps_f)
    nc.sync.dma_start(out=out, in_=res)


_CACHED_NC = None


def _get_nc():
    global _CACHED_NC
    if _CACHED_NC is None:
        nc = bacc.Bacc("TRN2", target_bir_lowering=False, debug=False)
        pred = nc.dram_tensor("predicted", [N, D], F32, kind="ExternalInput").ap()
        exp = nc.dram_tensor("expected", [N, D], F32, kind="ExternalInput").ap()
        out = nc.dram_tensor("out", [1, 1], F32, kind="ExternalOutput").ap()
        with tile.TileContext(nc) as tc, ExitStack() as ctx:
            _build_chamfer(ctx, nc, tc, pred, exp, out)
        nc.compile()
        _CACHED_NC = nc
    return _CACHED_NC


def run_spmd(predicted, expected, **kwargs):
    """Run on the 8 cores; returns (loss[8], BassKernelResults)."""
    predicted = np.asarray(predicted, dtype=np.float32)
    expected = np.asarray(expected, dtype=np.float32)
    assert predicted.shape == (M, N, D) and expected.shape == (M, N, D)
    nc = _get_nc()
    in_maps = [
        {
            "predicted": np.ascontiguousarray(predicted[m]),
            "expected": np.ascontiguousarray(expected[m]),
        }
        for m in range(M)
    ]
    res = bass_utils.run_bass_kernel_spmd(nc, in_maps, core_ids=list(range(M)), **kwargs)
    loss = np.array([res.results[m]["out"][0, 0] for m in range(M)], dtype=np.float32)
    return loss, res


def kernel(predicted, expected):
    loss, _ = run_spmd(predicted, expected)
    return loss


# revision 27
# speedup vs baseline: 2671.0124x; 2671.0124x over previous
"""Chamfer loss kernel for Trainium2 (Bass/Tile), 8 NeuronCores.

Problem: predicted/expected [M=8, N=4096, D=64] fp32; per batch element m:
    dist[n,k] = ||a_n||^2 + ||b_k||^2 - 2 a_n.b_k   (clamped at 0)
    loss[m] = sum_n min_k dist + sum_k min_n dist

Sharding: pure data parallel over m — one batch element per core.

Per-core algorithm:
  - Build transposed operands aT/bT [D, N] in fp32r via PE transposes.
  - Append a 66th/67th contraction row pair: ones on the a side,
    (-|b|^2/2) split hi/lo on the b side, so the K=66 matmul produces
    psum = a.b - |b|^2/2 directly.
  - ACT copies each psum batch to SBUF fp16 while adding a per-partition
    bias of -|a|^2/2, giving copy_t = -dist/2.  Both chamfer directions
    are then max-reductions of copy_t:
      * rowmax (min over k): DVE tensor_scalar max-accum (4x mode)
      * colmax (min over n): DVE tensor_max accumulation (2x mode)
  - Epilogue: relu(-2*max) (= clamped min distance) summed per partition
    via ACT accum, cross-partition summed via a ones-matmul.
"""

from contextlib import ExitStack

import numpy as np

import concourse.bacc as bacc
import concourse.bass_utils as bass_utils
import concourse.mybir as mybir
import concourse.tile as tile
from concourse.masks import make_identity

M, N, D = 8, 4096, 64
P = 128  # partitions / a-row tile
CH = 512  # b-column chunk (one psum bank)
GRP = 1024  # columns per psum batch (2 banks)
NT = N // P  # 32 a-row tiles
NG = N // GRP  # 2 column groups per row tile
KAUG = D + 2  # contraction rows incl. ones x (-b2/2) hi/lo

F32 = mybir.dt.float32
F32R = mybir.dt.float32r
F16 = mybir.dt.float16
NEG_BIG = -3.0e38  # fp32 immediates
F16_NEG = -60000.0  # fp16-safe "-inf"
ALU_MAX = mybir.AluOpType.max
DVE_DIRECT_MOD = 14  # every k-th batch: fused DVE path instead of ACT copy
GPS_MOD = 2  # every k-th batch: colmax update on GPSIMD


def _build_chamfer(ctx, nc, tc, pred, exp, out, reps=1):
    fx = ctx.enter_context(tc.tile_pool(name="fx", bufs=1))
    work = ctx.enter_context(tc.tile_pool(name="work", bufs=2))
    cpool = ctx.enter_context(tc.tile_pool(name="cp", bufs=4))
    spool = ctx.enter_context(tc.tile_pool(name="sp", bufs=4))
    ps = ctx.enter_context(tc.tile_pool(name="ps", bufs=(2 if GRP >= 2048 else 4), space="PSUM"))

    ident = fx.tile([P, P], F32, tag="ident")
    make_identity(nc, ident)

    prev_res = None
    for rep in range(reps):
        prev_res = _chamfer_once(
            nc, fx, work, cpool, spool, ps, ident, pred, exp, out, rep, prev_res
        )


def _chamfer_once(nc, fx, work, cpool, spool, ps, ident, pred, exp, out, rep,
                  prev_res):
    # ---- load inputs in natural layout [128, 32, 64], split in halves so
    # the norm/transpose pipelines start after the first half lands ----
    a_nat = fx.tile([P, NT, D], F32, tag="a_nat", name=f"a_nat_{rep}")
    b_nat = fx.tile([P, NT, D], F32, tag="b_nat", name=f"b_nat_{rep}")
    pred_v = pred.rearrange("(p c) d -> p c d", p=P)
    exp_v = exp.rearrange("(p c) d -> p c d", p=P)
    HC = NT // 2
    for h in range(2):
        cs = slice(h * HC, (h + 1) * HC)
        nc.sync.dma_start(out=b_nat[:, cs, :], in_=exp_v[:, cs, :])
        nc.sync.dma_start(out=a_nat[:, cs, :], in_=pred_v[:, cs, :])

    # independent early work: ones rows staging, fp16 identity
    aT = fx.tile([KAUG, N], F32R, tag="aT")
    bT = fx.tile([KAUG, N], F32R, tag="bT")
    ones_row = work.tile([2, N], F32, tag="ones_row")
    nc.gpsimd.memset(ones_row, 1.0)
    nc.vector.tensor_copy(aT[D:D + 2, :], ones_row)
    ident16 = fx.tile([P, P], F16, tag="ident16")
    nc.vector.tensor_copy(ident16, ident)
    colmax = [
        fx.tile([P, N], F16, tag=f"colmax{i}", name=f"colmax{i}_{rep}")
        for i in range(2)
    ]
    nc.vector.memset(colmax[0], F16_NEG)

    # ---- squared norms (per input half) ----
    # a2 stays in column layout [128, 32]: it is the ACT bias source (-a2/2).
    a2 = fx.tile([P, NT], F32, tag="a2")
    neg_a2_half = fx.tile([P, NT], F32, tag="na2")
    sq_a = work.tile([P, NT * D], F32, tag="sq_a")
    b2 = fx.tile([P, NT], F32, tag="b2")
    sq_b = work.tile([P, NT * D], F32, tag="sq_b")
    TG = GRP // P  # chunks per transpose/copy group (psum tile = GRP wide)
    GPH = NT // TG // 2  # transpose groups per input half

    def data_transposes(src, dst, h, on_act):
        for g in range(h * GPH, (h + 1) * GPH):
            pt = ps.tile([D, GRP], F32, tag="mm",
                         name=f"ptr_{rep}_{dst.name}_{g}")
            for c in range(TG):
                nc.tensor.transpose(
                    pt[:, c * P:(c + 1) * P], src[:, TG * g + c, :], ident
                )
            if on_act:
                nc.scalar.copy(dst[0:D, g * GRP:(g + 1) * GRP], pt)
            else:
                nc.vector.tensor_copy(dst[0:D, g * GRP:(g + 1) * GRP], pt)

    for h in range(2):
        cs = slice(h * HC, (h + 1) * HC)
        fs = slice(h * HC * D, (h + 1) * HC * D)
        # b-data transposes first: they only need the input half
        data_transposes(b_nat, bT, h, on_act=False)
        # the b2 row gates every matmul: keep its chain dense on ACT
        nc.scalar.square(sq_b[:, fs], b_nat[:, cs, :].rearrange("p c d -> p (c d)"))
        nc.vector.reduce_sum(b2[:, cs],
                             sq_b[:, fs].rearrange("p (c d) -> p c d", d=D),
                             axis=mybir.AxisListType.X)
        ps_b2 = ps.tile([HC, P], F32, tag="mm", name=f"psb2_{rep}_{h}")
        nc.tensor.transpose(ps_b2, b2[:, cs], ident)
        nb2h = fx.tile([HC, P], F32, tag=f"nb2h{h}", name=f"nb2h_{rep}_{h}")
        nb2h_r = fx.tile([HC, P], F32R, tag=f"nb2hr{h}", name=f"nb2hr_{rep}_{h}")
        nb2l_r = fx.tile([HC, P], F32R, tag=f"nb2lr{h}", name=f"nb2lr_{rep}_{h}")
        nc.scalar.mul(nb2h, ps_b2, -0.5)
        nc.scalar.copy(nb2h_r, nb2h)
        nc.vector.tensor_sub(nb2l_r, nb2h, nb2h_r.bitcast(F32))
        nc.sync.dma_start(
            out=bT[D:D + 1, h * HC * P:(h + 1) * HC * P].rearrange(
                "o (c f) -> o c f", c=HC),
            in_=nb2h_r,
        )
        nc.sync.dma_start(
            out=bT[D + 1:D + 2, h * HC * P:(h + 1) * HC * P].rearrange(
                "o (c f) -> o c f", c=HC),
            in_=nb2l_r,
        )
        data_transposes(a_nat, aT, h, on_act=True)
        nc.scalar.square(sq_a[:, fs], a_nat[:, cs, :].rearrange("p c d -> p (c d)"))
        nc.vector.reduce_sum(a2[:, cs],
                             sq_a[:, fs].rearrange("p (c d) -> p c d", d=D),
                             axis=mybir.AxisListType.X)
        nc.scalar.mul(neg_a2_half[:, cs], a2[:, cs], -0.5)
    if prev_res is not None:
        # timing builds only: chain reps (x0 keeps values identical but
        # defeats DCE and serializes rep r+1 behind rep r's result)
        scaled0 = fx.tile([1, 1], F32, tag="scaled0", name=f"scaled0_{rep}")
        nc.vector.tensor_scalar_mul(scaled0, prev_res, 0.0)
        nc.vector.tensor_add(
            neg_a2_half[0:1, 0:1], neg_a2_half[0:1, 0:1], scaled0
        )

    # ---- accumulators ----
    rowparts = fx.tile([P, NT, NG], F32, tag="rowparts")

    # ---- main loop ----
    # Engine mix: most batches use ACT for the psum->fp16 copy (+bias) and
    # DVE for rowmax; every DVE_DIRECT_EVERY-th batch instead uses a single
    # fused DVE tensor_scalar from PSUM (relieving ACT), and a fraction of
    # colmax updates run on the otherwise-idle GPSIMD.
    t = 0
    for i in range(NT):
        lhsT = aT[:, i * P:(i + 1) * P]
        src_cm = colmax[i % 2]
        dst_cm = colmax[(i + 1) % 2]
        for g in range(NG):
            pt = ps.tile([P, GRP], F32, tag="mm")
            for u in range(GRP // CH):
                rhs = bT[:, g * GRP + u * CH:g * GRP + (u + 1) * CH]
                nc.tensor.matmul(
                    pt[:, u * CH:(u + 1) * CH], lhsT, rhs, start=True, stop=True
                )
            ct = cpool.tile([P, GRP], F16, tag="ct")
            if t % DVE_DIRECT_MOD == 3:
                # fused: ct = psum + (-a2/2); rowmax accum. Off-ACT.
                nc.vector.tensor_scalar(
                    out=ct, in0=pt, scalar1=neg_a2_half[:, i:i + 1], scalar2=None,
                    op0=mybir.AluOpType.add, op1=ALU_MAX,
                    accum_out=rowparts[:, i, g:g + 1],
                )
            else:
                nc.scalar.activation(
                    out=ct, in_=pt, func=mybir.ActivationFunctionType.Identity,
                    bias=neg_a2_half[:, i:i + 1], scale=1.0,
                )
                sc = spool.tile([P, GRP], F16, tag="sc")
                nc.vector.tensor_scalar(
                    out=sc, in0=ct, scalar1=NEG_BIG, scalar2=None,
                    op0=ALU_MAX, op1=ALU_MAX, accum_out=rowparts[:, i, g:g + 1],
                )
            eng = nc.vector
            eng.tensor_max(
                dst_cm[:, g * GRP:(g + 1) * GRP], ct, src_cm[:, g * GRP:(g + 1) * GRP]
            )
            t += 1

    colmax_fin = colmax[NT % 2]

    # ---- epilogue: loss_ab from rowparts ----
    rmax = fx.tile([P, NT], F32, tag="rmax")
    nc.vector.reduce_max(rmax, rowparts, axis=mybir.AxisListType.X)
    relu_r = fx.tile([P, NT], F32, tag="relu_r")
    row_sum = fx.tile([P, 1], F32, tag="row_sum")
    nc.scalar.activation(
        out=relu_r, in_=rmax, func=mybir.ActivationFunctionType.Relu,
        scale=-2.0, accum_out=row_sum,
    )

    # ---- epilogue: loss_ba from colmax (cross-partition via PE transpose) ----
    colT = fx.tile([P, NT], F32, tag="colT")
    for gr in range(8):
        pt = ps.tile([P, 4 * P], F16, tag="mm", name=f"ept_{rep}_{gr}")
        for u in range(4):
            blk = 4 * gr + u
            nc.tensor.transpose(
                pt[:, u * P:(u + 1) * P], colmax_fin[:, blk * P:(blk + 1) * P],
                ident16,
            )
        nc.vector.reduce_max(
            colT[:, 4 * gr:4 * gr + 4],
            pt.rearrange("p (u f) -> p u f", u=4),
            axis=mybir.AxisListType.X,
        )
    relu_c = fx.tile([P, NT], F32, tag="relu_c")
    col_sum = fx.tile([P, 1], F32, tag="col_sum")
    nc.scalar.activation(
        out=relu_c, in_=colT, func=mybir.ActivationFunctionType.Relu,
        scale=-2.0, accum_out=col_sum,
    )

    # ---- total: cross-partition sum via ones-matmul ----
    both = fx.tile([P, 1], F32, tag="both")
    nc.vector.tensor_add(both, row_sum, col_sum)
    ones = fx.tile([P, 1], F32, tag="ones")
    nc.gpsimd.memset(ones, 1.0)
    ps_f = ps.tile([1, 1], F32, tag="mm")
    nc.tensor.matmul(ps_f, both, ones, start=True, stop=True)
    res = fx.tile([1, 1], F32, tag="res")
    nc.scalar.copy(res, ps_f)
    nc.sync.dma_start(out=out, in_=res)
    return res


_CACHED_NC = None


def _get_nc():
    global _CACHED_NC
    if _CACHED_NC is None:
        nc = bacc.Bacc("TRN2", target_bir_lowering=False, debug=False)
        pred = nc.dram_tensor("predicted", [N, D], F32, kind="ExternalInput").ap()
        exp = nc.dram_tensor("expected", [N, D], F32, kind="ExternalInput").ap()
        out = nc.dram_tensor("out", [1, 1], F32, kind="ExternalOutput").ap()
        with tile.TileContext(nc) as tc, ExitStack() as ctx:
            _build_chamfer(ctx, nc, tc, pred, exp, out)
        nc.compile()
        _CACHED_NC = nc
    return _CACHED_NC


def run_spmd(predicted, expected, **kwargs):
    """Run on the 8 cores; returns (loss[8], BassKernelResults)."""
    predicted = np.asarray(predicted, dtype=np.float32)
    expected = np.asarray(expected, dtype=np.float32)
    assert predicted.shape == (M, N, D) and expected.shape == (M, N, D)
    nc = _get_nc()
    in_maps = [
        {
            "predicted": np.ascontiguousarray(predicted[m]),
            "expected": np.ascontiguousarray(expected[m]),
        }
        for m in range(M)
    ]
    res = bass_utils.run_bass_kernel_spmd(nc, in_maps, core_ids=list(range(M)), **kwargs)
    loss = np.array([res.results[m]["out"][0, 0] for m in range(M)], dtype=np.float32)
    return loss, res


def kernel(predicted, expected):
    loss, _ = run_spmd(predicted, expected)
    return loss


# revision 33
# speedup vs baseline: 2766.3638x; 1.0357x over previous
"""Chamfer loss kernel for Trainium2 (Bass/Tile), 8 NeuronCores.

Problem: predicted/expected [M=8, N=4096, D=64] fp32; per batch element m:
    dist[n,k] = ||a_n||^2 + ||b_k||^2 - 2 a_n.b_k   (clamped at 0)
    loss[m] = sum_n min_k dist + sum_k min_n dist

Sharding: pure data parallel over m — one batch element per core.

Per-core algorithm:
  - Build transposed operands aT/bT [D, N] in fp32r via PE transposes.
  - Append a 66th/67th contraction row pair: ones on the a side,
    (-|b|^2/2) split hi/lo on the b side, so the K=66 matmul produces
    psum = a.b - |b|^2/2 directly.
  - ACT copies each psum batch to SBUF fp16 while adding a per-partition
    bias of -|a|^2/2, giving copy_t = -dist/2.  Both chamfer directions
    are then max-reductions of copy_t:
      * rowmax (min over k): DVE tensor_scalar max-accum (4x mode)
      * colmax (min over n): DVE tensor_max accumulation (2x mode)
  - Epilogue: relu(-2*max) (= clamped min distance) summed per partition
    via ACT accum, cross-partition summed via a ones-matmul.
"""

from contextlib import ExitStack

import numpy as np

import concourse.bacc as bacc
import concourse.bass_utils as bass_utils
import concourse.mybir as mybir
import concourse.tile as tile
from concourse.masks import make_identity

M, N, D = 8, 4096, 64
P = 128  # partitions / a-row tile
CH = 512  # b-column chunk (one psum bank)
GRP = 1024  # columns per psum batch (2 banks)
NT = N // P  # 32 a-row tiles
NG = N // GRP  # 2 column groups per row tile
KAUG = D + 2  # contraction rows incl. ones x (-b2/2) hi/lo

F32 = mybir.dt.float32
F32R = mybir.dt.float32r
F16 = mybir.dt.float16
NEG_BIG = -3.0e38  # fp32 immediates
F16_NEG = -60000.0  # fp16-safe "-inf"
ALU_MAX = mybir.AluOpType.max
DVE_DIRECT_MOD = 12  # every k-th batch: fused DVE path instead of ACT copy
GPS_MOD = 2  # every k-th batch: colmax update on GPSIMD


def _build_chamfer(ctx, nc, tc, pred, exp, out, reps=1):
    fx = ctx.enter_context(tc.tile_pool(name="fx", bufs=1))
    work = ctx.enter_context(tc.tile_pool(name="work", bufs=2))
    cpool = ctx.enter_context(tc.tile_pool(name="cp", bufs=4))
    spool = ctx.enter_context(tc.tile_pool(name="sp", bufs=4))
    ps = ctx.enter_context(tc.tile_pool(name="ps", bufs=(2 if GRP >= 2048 else 4), space="PSUM"))

    ident = fx.tile([P, P], F32, tag="ident")
    make_identity(nc, ident)

    prev_res = None
    for rep in range(reps):
        prev_res = _chamfer_once(
            nc, fx, work, cpool, spool, ps, ident, pred, exp, out, rep, prev_res
        )


def _chamfer_once(nc, fx, work, cpool, spool, ps, ident, pred, exp, out, rep,
                  prev_res):
    # ---- load inputs in natural layout [128, 32, 64], split in halves so
    # the norm/transpose pipelines start after the first half lands ----
    a_nat = fx.tile([P, NT, D], F32, tag="a_nat", name=f"a_nat_{rep}")
    b_nat = fx.tile([P, NT, D], F32, tag="b_nat", name=f"b_nat_{rep}")
    pred_v = pred.rearrange("(p c) d -> p c d", p=P)
    exp_v = exp.rearrange("(p c) d -> p c d", p=P)
    HC = NT // 2
    for h in range(2):
        cs = slice(h * HC, (h + 1) * HC)
        nc.sync.dma_start(out=b_nat[:, cs, :], in_=exp_v[:, cs, :])
        nc.sync.dma_start(out=a_nat[:, cs, :], in_=pred_v[:, cs, :])

    # independent early work: ones rows staging, fp16 identity
    aT = fx.tile([KAUG, N], F32R, tag="aT")
    bT = fx.tile([KAUG, N], F32R, tag="bT")
    ones_row = work.tile([2, N], F32, tag="ones_row")
    nc.gpsimd.memset(ones_row, 1.0)
    nc.gpsimd.tensor_copy(aT[D:D + 2, :], ones_row)
    ident16 = fx.tile([P, P], F16, tag="ident16")
    nc.vector.tensor_copy(ident16, ident)
    colmax = [
        fx.tile([P, N], F16, tag=f"colmax{i}", name=f"colmax{i}_{rep}")
        for i in range(2)
    ]
    nc.gpsimd.memset(colmax[0], F16_NEG)

    # ---- squared norms (per input half) ----
    # a2 stays in column layout [128, 32]: it is the ACT bias source (-a2/2).
    a2 = fx.tile([P, NT], F32, tag="a2")
    neg_a2_half = fx.tile([P, NT], F32, tag="na2")
    sq_a = work.tile([P, NT * D], F32, tag="sq_a")
    b2 = fx.tile([P, NT], F32, tag="b2")
    sq_b = work.tile([P, NT * D], F32, tag="sq_b")
    TG = GRP // P  # chunks per transpose/copy group (psum tile = GRP wide)
    GPH = NT // TG // 2  # transpose groups per input half

    def data_transposes(src, dst, h, on_act):
        for g in range(h * GPH, (h + 1) * GPH):
            pt = ps.tile([D, GRP], F32, tag="mm",
                         name=f"ptr_{rep}_{dst.name}_{g}")
            for c in range(TG):
                nc.tensor.transpose(
                    pt[:, c * P:(c + 1) * P], src[:, TG * g + c, :], ident
                )
            if on_act:
                nc.scalar.copy(dst[0:D, g * GRP:(g + 1) * GRP], pt)
            else:
                nc.vector.tensor_copy(dst[0:D, g * GRP:(g + 1) * GRP], pt)

    for h in range(2):
        cs = slice(h * HC, (h + 1) * HC)
        fs = slice(h * HC * D, (h + 1) * HC * D)
        # b-data transposes first: they only need the input half
        data_transposes(b_nat, bT, h, on_act=False)
        # the b2 row gates every matmul: keep its chain dense on ACT
        nc.scalar.square(sq_b[:, fs], b_nat[:, cs, :].rearrange("p c d -> p (c d)"))
        nc.vector.reduce_sum(b2[:, cs],
                             sq_b[:, fs].rearrange("p (c d) -> p c d", d=D),
                             axis=mybir.AxisListType.X)
        ps_b2 = ps.tile([HC, P], F32, tag="mm", name=f"psb2_{rep}_{h}")
        nc.tensor.transpose(ps_b2, b2[:, cs], ident)
        nb2h = fx.tile([HC, P], F32, tag=f"nb2h{h}", name=f"nb2h_{rep}_{h}")
        nb2h_r = fx.tile([HC, P], F32R, tag=f"nb2hr{h}", name=f"nb2hr_{rep}_{h}")
        nb2l_r = fx.tile([HC, P], F32R, tag=f"nb2lr{h}", name=f"nb2lr_{rep}_{h}")
        nc.scalar.mul(nb2h, ps_b2, -0.5)
        nc.scalar.copy(nb2h_r, nb2h)
        nc.vector.tensor_sub(nb2l_r, nb2h, nb2h_r.bitcast(F32))
        nc.sync.dma_start(
            out=bT[D:D + 1, h * HC * P:(h + 1) * HC * P].rearrange(
                "o (c f) -> o c f", c=HC),
            in_=nb2h_r,
        )
        nc.sync.dma_start(
            out=bT[D + 1:D + 2, h * HC * P:(h + 1) * HC * P].rearrange(
                "o (c f) -> o c f", c=HC),
            in_=nb2l_r,
        )
        data_transposes(a_nat, aT, h, on_act=True)
        nc.scalar.square(sq_a[:, fs], a_nat[:, cs, :].rearrange("p c d -> p (c d)"))
        nc.vector.reduce_sum(a2[:, cs],
                             sq_a[:, fs].rearrange("p (c d) -> p c d", d=D),
                             axis=mybir.AxisListType.X)
        nc.scalar.mul(neg_a2_half[:, cs], a2[:, cs], -0.5)
    if prev_res is not None:
        # timing builds only: chain reps (x0 keeps values identical but
        # defeats DCE and serializes rep r+1 behind rep r's result)
        scaled0 = fx.tile([1, 1], F32, tag="scaled0", name=f"scaled0_{rep}")
        nc.vector.tensor_scalar_mul(scaled0, prev_res, 0.0)
        nc.vector.tensor_add(
            neg_a2_half[0:1, 0:1], neg_a2_half[0:1, 0:1], scaled0
        )

    # ---- accumulators ----
    rowparts = fx.tile([P, NT, NG], F32, tag="rowparts")

    # ---- main loop ----
    # Pairs of 1024-col groups share one [128,2048] fp16 tile so the DVE
    # colmax/rowmax ops run at FD=2048 (halving per-op overhead). Most
    # copies ride ACT (with the -a2/2 bias); every DVE_DIRECT_MOD-th batch
    # uses a fused DVE tensor_scalar straight from PSUM to relieve ACT
    # (breaking that pair's shared rowmax only).
    # Unwritten rowparts slots read as NEG_BIG (set below) so the final
    # reduce over slots ignores them.
    nc.vector.memset(rowparts, NEG_BIG)
    colT = fx.tile([P, NT], F32, tag="colT")
    t = 0
    for pp in range(NG // 2):
        for i in range(NT):
            lhsT = aT[:, i * P:(i + 1) * P]
            src_cm = colmax[i % 2]
            dst_cm = colmax[(i + 1) % 2]
            ct2 = cpool.tile([P, 2 * GRP], F16, tag="ct")
            direct = [False, False]
            for m in range(2):
                g = 2 * pp + m
                pt = ps.tile([P, GRP], F32, tag="mm")
                for u in range(GRP // CH):
                    rhs = bT[:, g * GRP + u * CH:g * GRP + (u + 1) * CH]
                    nc.tensor.matmul(
                        pt[:, u * CH:(u + 1) * CH], lhsT, rhs,
                        start=True, stop=True,
                    )
                cth = ct2[:, m * GRP:(m + 1) * GRP]
                if t % DVE_DIRECT_MOD == 3:
                    direct[m] = True
                    # fused: ct half = psum + (-a2/2); own rowmax accum
                    nc.vector.tensor_scalar(
                        out=cth, in0=pt, scalar1=neg_a2_half[:, i:i + 1],
                        scalar2=None, op0=mybir.AluOpType.add, op1=ALU_MAX,
                        accum_out=rowparts[:, i, g:g + 1],
                    )
                else:
                    nc.scalar.activation(
                        out=cth, in_=pt,
                        func=mybir.ActivationFunctionType.Identity,
                        bias=neg_a2_half[:, i:i + 1], scale=1.0,
                    )
                t += 1
            cols = slice(2 * pp * GRP, 2 * (pp + 1) * GRP)
            nc.vector.tensor_max(dst_cm[:, cols], ct2, src_cm[:, cols])
            if not (direct[0] or direct[1]):
                sc = spool.tile([P, 2 * GRP], F16, tag="sc")
                nc.vector.tensor_scalar(
                    out=sc, in0=ct2, scalar1=NEG_BIG, scalar2=None,
                    op0=ALU_MAX, op1=ALU_MAX,
                    accum_out=rowparts[:, i, 2 * pp:2 * pp + 1],
                )
            else:
                for m in range(2):
                    if direct[m]:
                        continue
                    g = 2 * pp + m
                    sc = spool.tile([P, 2 * GRP], F16, tag="sc")
                    nc.vector.tensor_scalar(
                        out=sc[:, 0:GRP], in0=ct2[:, m * GRP:(m + 1) * GRP],
                        scalar1=NEG_BIG, scalar2=None,
                        op0=ALU_MAX, op1=ALU_MAX,
                        accum_out=rowparts[:, i, g:g + 1],
                    )

    colmax_fin = colmax[NT % 2]

    # ---- epilogue: loss_ab from rowparts ----
    rmax = fx.tile([P, NT], F32, tag="rmax")
    nc.vector.reduce_max(rmax, rowparts, axis=mybir.AxisListType.X)
    relu_r = fx.tile([P, NT], F32, tag="relu_r")
    row_sum = fx.tile([P, 1], F32, tag="row_sum")
    nc.scalar.activation(
        out=relu_r, in_=rmax, func=mybir.ActivationFunctionType.Relu,
        scale=-2.0, accum_out=row_sum,
    )

    # ---- epilogue: loss_ba from colmax (cross-partition via PE transpose) ----
    for gr in range(8):
        ept = ps.tile([P, 4 * P], F16, tag="mm", name=f"ept_{rep}_{gr}")
        for u in range(4):
            blk = 4 * gr + u
            nc.tensor.transpose(
                ept[:, u * P:(u + 1) * P], colmax_fin[:, blk * P:(blk + 1) * P],
                ident16,
            )
        nc.vector.reduce_max(
            colT[:, 4 * gr:4 * gr + 4],
            ept.rearrange("p (u f) -> p u f", u=4),
            axis=mybir.AxisListType.X,
        )
    relu_c = fx.tile([P, NT], F32, tag="relu_c")
    col_sum = fx.tile([P, 1], F32, tag="col_sum")
    nc.scalar.activation(
        out=relu_c, in_=colT, func=mybir.ActivationFunctionType.Relu,
        scale=-2.0, accum_out=col_sum,
    )

    # ---- total: cross-partition sum via ones-matmul ----
    both = fx.tile([P, 1], F32, tag="both")
    nc.vector.tensor_add(both, row_sum, col_sum)
    ones = fx.tile([P, 1], F32, tag="ones")
    nc.gpsimd.memset(ones, 1.0)
    ps_f = ps.tile([1, 1], F32, tag="mm")
    nc.tensor.matmul(ps_f, both, ones, start=True, stop=True)
    res = fx.tile([1, 1], F32, tag="res")
    nc.scalar.copy(res, ps_f)
    nc.sync.dma_start(out=out, in_=res)
    return res


_CACHED_NC = None


def _get_nc():
    global _CACHED_NC
    if _CACHED_NC is None:
        nc = bacc.Bacc("TRN2", target_bir_lowering=False, debug=False)
        pred = nc.dram_tensor("predicted", [N, D], F32, kind="ExternalInput").ap()
        exp = nc.dram_tensor("expected", [N, D], F32, kind="ExternalInput").ap()
        out = nc.dram_tensor("out", [1, 1], F32, kind="ExternalOutput").ap()
        with tile.TileContext(nc) as tc, ExitStack() as ctx:
            _build_chamfer(ctx, nc, tc, pred, exp, out)
        nc.compile()
        _CACHED_NC = nc
    return _CACHED_NC


def run_spmd(predicted, expected, **kwargs):
    """Run on the 8 cores; returns (loss[8], BassKernelResults)."""
    predicted = np.asarray(predicted, dtype=np.float32)
    expected = np.asarray(expected, dtype=np.float32)
    assert predicted.shape == (M, N, D) and expected.shape == (M, N, D)
    nc = _get_nc()
    in_maps = [
        {
            "predicted": np.ascontiguousarray(predicted[m]),
            "expected": np.ascontiguousarray(expected[m]),
        }
        for m in range(M)
    ]
    res = bass_utils.run_bass_kernel_spmd(nc, in_maps, core_ids=list(range(M)), **kwargs)
    loss = np.array([res.results[m]["out"][0, 0] for m in range(M)], dtype=np.float32)
    return loss, res


def kernel(predicted, expected):
    loss, _ = run_spmd(predicted, expected)
    return loss


# revision 36
# speedup vs baseline: 2767.4673x; 1.0004x over previous
"""Chamfer loss kernel for Trainium2 (Bass/Tile), 8 NeuronCores.

Problem: predicted/expected [M=8, N=4096, D=64] fp32; per batch element m:
    dist[n,k] = ||a_n||^2 + ||b_k||^2 - 2 a_n.b_k   (clamped at 0)
    loss[m] = sum_n min_k dist + sum_k min_n dist

Sharding: pure data parallel over m — one batch element per core.

Per-core algorithm:
  - Build transposed operands aT/bT [D, N] in fp32r via PE transposes.
  - Append a 66th/67th contraction row pair: ones on the a side,
    (-|b|^2/2) split hi/lo on the b side, so the K=66 matmul produces
    psum = a.b - |b|^2/2 directly.
  - ACT copies each psum batch to SBUF fp16 while adding a per-partition
    bias of -|a|^2/2, giving copy_t = -dist/2.  Both chamfer directions
    are then max-reductions of copy_t:
      * rowmax (min over k): DVE tensor_scalar max-accum (4x mode)
      * colmax (min over n): DVE tensor_max accumulation (2x mode)
  - Epilogue: relu(-2*max) (= clamped min distance) summed per partition
    via ACT accum, cross-partition summed via a ones-matmul.
"""

from contextlib import ExitStack

import numpy as np

import concourse.bacc as bacc
import concourse.bass_utils as bass_utils
import concourse.mybir as mybir
import concourse.tile as tile
from concourse.masks import make_identity

M, N, D = 8, 4096, 64
P = 128  # partitions / a-row tile
CH = 512  # b-column chunk (one psum bank)
GRP = 1024  # columns per psum batch (2 banks)
NT = N // P  # 32 a-row tiles
NG = N // GRP  # 2 column groups per row tile
KAUG = D + 2  # contraction rows incl. ones x (-b2/2) hi/lo

F32 = mybir.dt.float32
F32R = mybir.dt.float32r
F16 = mybir.dt.float16
NEG_BIG = -3.0e38  # fp32 immediates
F16_NEG = -60000.0  # fp16-safe "-inf"
ALU_MAX = mybir.AluOpType.max
DVE_DIRECT_MOD = 12  # every k-th batch: fused DVE path instead of ACT copy
GPS_MOD = 2  # every k-th batch: colmax update on GPSIMD


def _build_chamfer(ctx, nc, tc, pred, exp, out, reps=1):
    fx = ctx.enter_context(tc.tile_pool(name="fx", bufs=1))
    work = ctx.enter_context(tc.tile_pool(name="work", bufs=2))
    cpool = ctx.enter_context(tc.tile_pool(name="cp", bufs=4))
    spool = ctx.enter_context(tc.tile_pool(name="sp", bufs=4))
    ps = ctx.enter_context(tc.tile_pool(name="ps", bufs=(2 if GRP >= 2048 else 3), space="PSUM"))

    ident = fx.tile([P, P], F32, tag="ident")
    make_identity(nc, ident)

    prev_res = None
    for rep in range(reps):
        prev_res = _chamfer_once(
            nc, fx, work, cpool, spool, ps, ident, pred, exp, out, rep, prev_res
        )


def _chamfer_once(nc, fx, work, cpool, spool, ps, ident, pred, exp, out, rep,
                  prev_res):
    # ---- load inputs in natural layout [128, 32, 64], split in halves so
    # the norm/transpose pipelines start after the first half lands ----
    a_nat = fx.tile([P, NT, D], F32, tag="a_nat", name=f"a_nat_{rep}")
    b_nat = fx.tile([P, NT, D], F32, tag="b_nat", name=f"b_nat_{rep}")
    pred_v = pred.rearrange("(p c) d -> p c d", p=P)
    exp_v = exp.rearrange("(p c) d -> p c d", p=P)
    HC = NT // 2
    for h in range(2):
        cs = slice(h * HC, (h + 1) * HC)
        nc.sync.dma_start(out=b_nat[:, cs, :], in_=exp_v[:, cs, :])
        nc.sync.dma_start(out=a_nat[:, cs, :], in_=pred_v[:, cs, :])

    # independent early work: ones rows staging, fp16 identity
    aT = fx.tile([KAUG, N], F32R, tag="aT")
    bT = fx.tile([KAUG, N], F32R, tag="bT")
    ones_row = work.tile([2, N], F32, tag="ones_row")
    nc.gpsimd.memset(ones_row, 1.0)
    nc.gpsimd.tensor_copy(aT[D:D + 2, :], ones_row)
    ident16 = fx.tile([P, P], F16, tag="ident16")
    nc.vector.tensor_copy(ident16, ident)
    colmax = [
        fx.tile([P, N], F16, tag=f"colmax{i}", name=f"colmax{i}_{rep}")
        for i in range(2)
    ]
    nc.gpsimd.memset(colmax[0], F16_NEG)

    # ---- squared norms (per input half) ----
    # a2 stays in column layout [128, 32]: it is the ACT bias source (-a2/2).
    a2 = fx.tile([P, NT], F32, tag="a2")
    neg_a2_half = fx.tile([P, NT], F32, tag="na2")
    sq_a = work.tile([P, NT * D], F32, tag="sq_a")
    b2 = fx.tile([P, NT], F32, tag="b2")
    sq_b = work.tile([P, NT * D], F32, tag="sq_b")
    TG = GRP // P  # chunks per transpose/copy group (psum tile = GRP wide)
    GPH = NT // TG // 2  # transpose groups per input half

    def data_transposes(src, dst, h, on_act):
        for g in range(h * GPH, (h + 1) * GPH):
            pt = ps.tile([D, GRP], F32, tag="mm",
                         name=f"ptr_{rep}_{dst.name}_{g}")
            for c in range(TG):
                nc.tensor.transpose(
                    pt[:, c * P:(c + 1) * P], src[:, TG * g + c, :], ident
                )
            if on_act:
                nc.scalar.copy(dst[0:D, g * GRP:(g + 1) * GRP], pt)
            else:
                nc.vector.tensor_copy(dst[0:D, g * GRP:(g + 1) * GRP], pt)

    for h in range(2):
        cs = slice(h * HC, (h + 1) * HC)
        fs = slice(h * HC * D, (h + 1) * HC * D)
        # b-data transposes first: they only need the input half
        data_transposes(b_nat, bT, h, on_act=False)
        # the b2 row gates every matmul: keep its chain dense on ACT
        nc.scalar.square(sq_b[:, fs], b_nat[:, cs, :].rearrange("p c d -> p (c d)"))
        nc.vector.reduce_sum(b2[:, cs],
                             sq_b[:, fs].rearrange("p (c d) -> p c d", d=D),
                             axis=mybir.AxisListType.X)
        ps_b2 = ps.tile([HC, P], F32, tag="mm", name=f"psb2_{rep}_{h}")
        nc.tensor.transpose(ps_b2, b2[:, cs], ident)
        nb2h = fx.tile([HC, P], F32, tag=f"nb2h{h}", name=f"nb2h_{rep}_{h}")
        nb2h_r = fx.tile([HC, P], F32R, tag=f"nb2hr{h}", name=f"nb2hr_{rep}_{h}")
        nb2l_r = fx.tile([HC, P], F32R, tag=f"nb2lr{h}", name=f"nb2lr_{rep}_{h}")
        nc.scalar.mul(nb2h, ps_b2, -0.5)
        nc.scalar.copy(nb2h_r, nb2h)
        nc.vector.tensor_sub(nb2l_r, nb2h, nb2h_r.bitcast(F32))
        nc.sync.dma_start(
            out=bT[D:D + 1, h * HC * P:(h + 1) * HC * P].rearrange(
                "o (c f) -> o c f", c=HC),
            in_=nb2h_r,
        )
        nc.sync.dma_start(
            out=bT[D + 1:D + 2, h * HC * P:(h + 1) * HC * P].rearrange(
                "o (c f) -> o c f", c=HC),
            in_=nb2l_r,
        )
        data_transposes(a_nat, aT, h, on_act=True)
        nc.scalar.square(sq_a[:, fs], a_nat[:, cs, :].rearrange("p c d -> p (c d)"))
        nc.vector.reduce_sum(a2[:, cs],
                             sq_a[:, fs].rearrange("p (c d) -> p c d", d=D),
                             axis=mybir.AxisListType.X)
        nc.scalar.mul(neg_a2_half[:, cs], a2[:, cs], -0.5)
    if prev_res is not None:
        # timing builds only: chain reps (x0 keeps values identical but
        # defeats DCE and serializes rep r+1 behind rep r's result)
        scaled0 = fx.tile([1, 1], F32, tag="scaled0", name=f"scaled0_{rep}")
        nc.vector.tensor_scalar_mul(scaled0, prev_res, 0.0)
        nc.vector.tensor_add(
            neg_a2_half[0:1, 0:1], neg_a2_half[0:1, 0:1], scaled0
        )

    # ---- accumulators ----
    rowparts = fx.tile([P, NT, NG], F32, tag="rowparts")

    # ---- main loop ----
    # Pairs of 1024-col groups share one [128,2048] fp16 tile so the DVE
    # colmax/rowmax ops run at FD=2048 (halving per-op overhead). Most
    # copies ride ACT (with the -a2/2 bias); every DVE_DIRECT_MOD-th batch
    # uses a fused DVE tensor_scalar straight from PSUM to relieve ACT
    # (breaking that pair's shared rowmax only).
    # Unwritten rowparts slots read as NEG_BIG (set below) so the final
    # reduce over slots ignores them.
    nc.vector.memset(rowparts, NEG_BIG)
    colT = fx.tile([P, NT], F32, tag="colT")
    t = 0
    for pp in range(NG // 2):
        for i in range(NT):
            lhsT = aT[:, i * P:(i + 1) * P]
            src_cm = colmax[i % 2]
            dst_cm = colmax[(i + 1) % 2]
            ct2 = cpool.tile([P, 2 * GRP], F16, tag="ct")
            direct = [False, False]
            for m in range(2):
                g = 2 * pp + m
                pt = ps.tile([P, GRP], F32, tag="mm")
                for u in range(GRP // CH):
                    rhs = bT[:, g * GRP + u * CH:g * GRP + (u + 1) * CH]
                    nc.tensor.matmul(
                        pt[:, u * CH:(u + 1) * CH], lhsT, rhs,
                        start=True, stop=True,
                    )
                cth = ct2[:, m * GRP:(m + 1) * GRP]
                if t % DVE_DIRECT_MOD == 3:
                    direct[m] = True
                    # fused: ct half = psum + (-a2/2); own rowmax accum
                    nc.vector.tensor_scalar(
                        out=cth, in0=pt, scalar1=neg_a2_half[:, i:i + 1],
                        scalar2=None, op0=mybir.AluOpType.add, op1=ALU_MAX,
                        accum_out=rowparts[:, i, g:g + 1],
                    )
                else:
                    nc.scalar.activation(
                        out=cth, in_=pt,
                        func=mybir.ActivationFunctionType.Identity,
                        bias=neg_a2_half[:, i:i + 1], scale=1.0,
                    )
                t += 1
            cols = slice(2 * pp * GRP, 2 * (pp + 1) * GRP)
            nc.vector.tensor_max(dst_cm[:, cols], ct2, src_cm[:, cols])
            if not (direct[0] or direct[1]):
                sc = spool.tile([P, 2 * GRP], F16, tag="sc")
                nc.vector.tensor_scalar(
                    out=sc, in0=ct2, scalar1=NEG_BIG, scalar2=None,
                    op0=ALU_MAX, op1=ALU_MAX,
                    accum_out=rowparts[:, i, 2 * pp:2 * pp + 1],
                )
            else:
                for m in range(2):
                    if direct[m]:
                        continue
                    g = 2 * pp + m
                    sc = spool.tile([P, 2 * GRP], F16, tag="sc")
                    nc.vector.tensor_scalar(
                        out=sc[:, 0:GRP], in0=ct2[:, m * GRP:(m + 1) * GRP],
                        scalar1=NEG_BIG, scalar2=None,
                        op0=ALU_MAX, op1=ALU_MAX,
                        accum_out=rowparts[:, i, g:g + 1],
                    )

    colmax_fin = colmax[NT % 2]

    # ---- epilogue: loss_ab from rowparts ----
    rmax = fx.tile([P, NT], F32, tag="rmax")
    nc.vector.reduce_max(rmax, rowparts, axis=mybir.AxisListType.X)
    relu_r = fx.tile([P, NT], F32, tag="relu_r")
    row_sum = fx.tile([P, 1], F32, tag="row_sum")
    nc.scalar.activation(
        out=relu_r, in_=rmax, func=mybir.ActivationFunctionType.Relu,
        scale=-2.0, accum_out=row_sum,
    )

    # ---- epilogue: loss_ba from colmax (cross-partition via PE transpose) ----
    for gr in range(8):
        ept = ps.tile([P, 4 * P], F16, tag="ep", bufs=2, name=f"ept_{rep}_{gr}")
        for u in range(4):
            blk = 4 * gr + u
            nc.tensor.transpose(
                ept[:, u * P:(u + 1) * P], colmax_fin[:, blk * P:(blk + 1) * P],
                ident16,
            )
        nc.vector.reduce_max(
            colT[:, 4 * gr:4 * gr + 4],
            ept.rearrange("p (u f) -> p u f", u=4),
            axis=mybir.AxisListType.X,
        )
    relu_c = fx.tile([P, NT], F32, tag="relu_c")
    col_sum = fx.tile([P, 1], F32, tag="col_sum")
    nc.scalar.activation(
        out=relu_c, in_=colT, func=mybir.ActivationFunctionType.Relu,
        scale=-2.0, accum_out=col_sum,
    )

    # ---- total: cross-partition sum via ones-matmul ----
    both = fx.tile([P, 1], F32, tag="both")
    nc.vector.tensor_add(both, row_sum, col_sum)
    ones = fx.tile([P, 1], F32, tag="ones")
    nc.gpsimd.memset(ones, 1.0)
    ps_f = ps.tile([1, 1], F32, tag="mm")
    nc.tensor.matmul(ps_f, both, ones, start=True, stop=True)
    res = fx.tile([1, 1], F32, tag="res")
    nc.scalar.copy(res, ps_f)
    nc.sync.dma_start(out=out, in_=res)
    return res


_CACHED_NC = None


def _get_nc():
    global _CACHED_NC
    if _CACHED_NC is None:
        nc = bacc.Bacc("TRN2", target_bir_lowering=False, debug=False)
        pred = nc.dram_tensor("predicted", [N, D], F32, kind="ExternalInput").ap()
        exp = nc.dram_tensor("expected", [N, D], F32, kind="ExternalInput").ap()
        out = nc.dram_tensor("out", [1, 1], F32, kind="ExternalOutput").ap()
        with tile.TileContext(nc) as tc, ExitStack() as ctx:
            _build_chamfer(ctx, nc, tc, pred, exp, out)
        nc.compile()
        _CACHED_NC = nc
    return _CACHED_NC


def run_spmd(predicted, expected, **kwargs):
    """Run on the 8 cores; returns (loss[8], BassKernelResults)."""
    predicted = np.asarray(predicted, dtype=np.float32)
    expected = np.asarray(expected, dtype=np.float32)
    assert predicted.shape == (M, N, D) and expected.shape == (M, N, D)
    nc = _get_nc()
    in_maps = [
        {
            "predicted": np.ascontiguousarray(predicted[m]),
            "expected": np.ascontiguousarray(expected[m]),
        }
        for m in range(M)
    ]
    res = bass_utils.run_bass_kernel_spmd(nc, in_maps, core_ids=list(range(M)), **kwargs)
    loss = np.array([res.results[m]["out"][0, 0] for m in range(M)], dtype=np.float32)
    return loss, res


def kernel(predicted, expected):
    loss, _ = run_spmd(predicted, expected)
    return loss
